# revision 65
# baseline (speedup 1.0000x reference)
"""Banded (sliding-window) multi-head attention on 8 Trainium2 NeuronCores.

Problem: B=2, S=2048, D=512, H=8 heads (hd=64), window=256 (|i-j| <= 128),
  qkv = x @ Wqkv + bqkv           -> per-head q,k,v
  scores = (q k^T masked to band) / 8 ; softmax ; out = (attn v) @ Wo + bo

Sharding: core = (batch b in {0,1}) x (head-group g in {0..3}); each core
computes 2 heads over the full sequence of one batch element plus the o_proj
partial product for its heads' embed slice. The host sums the 4 partials per
batch and adds bo.

v3: bf16 operands everywhere (fp32 PSUM accumulate), software-pipelined
emission (qkv chunk -> its V dma-transpose -> its scores blocks), o_proj
partials DMA'd straight from PSUM (no copy ops), V transposed by the DMA
xbar instead of the PE. GPSIMD cannot touch PSUM on TRN2, so the PSUM
reads split between ACT (exp, Q/K qkv copies) and DVE (V copies, recip,
fused normalize, band masks).
"""

import numpy as np
import ml_dtypes

import concourse.bass as bass  # noqa: F401  (engine types via nc)
import concourse.mybir as mybir
import concourse.tile as tile
from concourse import bacc
from concourse.bass_utils import run_bass_kernel_spmd

B, S, DIN, E = 2, 2048, 512, 512
H, HD = 8, 64
NB = S // 128      # 16 key/query blocks of 128
NCHUNK = S // 512  # 4 query chunks of 512
F32 = mybir.dt.float32
F32R = mybir.dt.float32r
BF16 = mybir.dt.bfloat16
EXPF = mybir.ActivationFunctionType.Exp
NPBF = ml_dtypes.bfloat16

# kb blocks whose 3-block score window is covered once qkv chunk cc is done
KB_GROUPS = {0: [0, 1, 2], 1: [3, 4, 5, 6], 2: [7, 8, 9, 10],
             3: [11, 12, 13, 14, 15]}

_CACHE = {}
LAST_RESULTS = None  # BassKernelResults of the most recent run (for test.py)


def _build_nc():
    nc = bacc.Bacc(None, target_bir_lowering=False, debug=False)

    xt = nc.dram_tensor("xt", [4, DIN, 512], BF16, kind="ExternalInput")
    wq = nc.dram_tensor("wq", [128, 4, 384], BF16, kind="ExternalInput")
    wo = nc.dram_tensor("wo", [128, E], BF16, kind="ExternalInput")
    km = nc.dram_tensor("km", [128, NB], F32, kind="ExternalInput")
    tm = nc.dram_tensor("tm", [128, 384], BF16, kind="ExternalInput")
    outt = nc.dram_tensor("outt", [E, S], BF16, kind="ExternalOutput")

    with tile.TileContext(nc) as tc:
        with (
            tc.tile_pool(name="sb", bufs=1) as sb,
            tc.tile_pool(name="ps_qkv", bufs=2, space="PSUM") as ps_qkv,
            tc.tile_pool(name="ps_st", bufs=2, space="PSUM") as ps_st,
            tc.tile_pool(name="ps_ot", bufs=2, space="PSUM") as ps_ot,
            tc.tile_pool(name="small", bufs=4) as small,
        ):
            xt_sb = sb.tile([128, 4, 4, 512], BF16)   # [p, kchunk, qchunk, q]
            wq_sb = sb.tile([128, 4, 384], BF16)      # [p, kchunk, fo]
            wo_sb = sb.tile([128, E], BF16)
            km_sb = sb.tile([128, NB], F32)
            tm2_sb = sb.tile([128, 384], BF16)
            qkvt = sb.tile([128, 2, S], BF16)         # fb0=Q, fb1=K (h0|h1)
            # V natural [key, v0|ones|v1|ones]: per head, [v_h | ones] is a
            # contiguous 128-col stationary read, so one AV matmul emits
            # values (rows 0:64) and denominator copies (rows 64:128)
            vboth = sb.tile([128, NB, 256], BF16)
            valst = sb.tile([128, S], BF16)           # normalized attn @ V
            outt_sb = sb.tile([128, 4, S], BF16)
            p_sb = sb.tile([128, NB, 2, 384], BF16)   # exp'd scores, h-minor

            # HWDGE descriptor generation (~630ns/op) serializes the input
            # stream, so only the launch-critical cc0 activations use it; the
            # weights, masks and later x chunks ride the SWDGE ring whose
            # descriptor prep runs on the idle-early Pool engine
            nc.gpsimd.dma_start(out=wq_sb, in_=wq[:, :, :])
            nc.gpsimd.dma_start(out=km_sb, in_=km[:, :])
            nc.gpsimd.dma_start(out=tm2_sb, in_=tm[:, :])
            for half in range(2):
                nc.sync.dma_start(
                    out=xt_sb[:, 2 * half:2 * half + 2, 0, :],
                    in_=xt[0, half * 256:(half + 1) * 256, :]
                    .rearrange("(kc p) q -> p kc q", p=128),
                )
            for cc in range(1, 4):
                nc.gpsimd.dma_start(
                    out=xt_sb[:, :, cc, :],
                    in_=xt[cc].rearrange("(kc p) q -> p kc q", p=128),
                )
            nc.sync.dma_start(out=wo_sb, in_=wo[:, :])

            # denominator ones blocks at cols 64:128 and 192:256 of every key
            # block, so AV emits 64 identical denominator rows (rows 64:128
            # of ot) and the normalize can read the reciprocal per-partition
            # -- no PE broadcast matmul needed
            nc.gpsimd.memset(vboth[:, :, 64:128], 1.0)
            nc.gpsimd.memset(vboth[:, :, 192:256], 1.0)

            def qkv_fb(cc, fb):
                # qkvT = Wg^T @ x[b]^T for query chunk cc, feature block fb;
                # Q/K copies on ACT (they gate the scores stream)
                ps = ps_qkv.tile([128, 512], F32, tag="qkv", name="ps")
                for kc in range(4):
                    nc.tensor.matmul(
                        ps,
                        wq_sb[:, kc, fb * 128:(fb + 1) * 128],
                        xt_sb[:, kc, cc, :],
                        start=(kc == 0),
                        stop=(kc == 3),
                    )
                nc.scalar.activation(
                    out=qkvt[:, fb, cc * 512:(cc + 1) * 512],
                    in_=ps,
                    func=mybir.ActivationFunctionType.Identity,
                )

            def qkv_fb_half(cc, fb, half):
                # 256-col half of a Q/K feature block (ramp shortener)
                o = half * 256
                ps = ps_qkv.tile([128, 256], F32, tag="qkv", name="psh")
                for kc in range(4):
                    nc.tensor.matmul(
                        ps,
                        wq_sb[:, kc, fb * 128:(fb + 1) * 128],
                        xt_sb[:, kc, cc, o:o + 256],
                        start=(kc == 0),
                        stop=(kc == 3),
                    )
                nc.scalar.activation(
                    out=qkvt[:, fb, cc * 512 + o:cc * 512 + o + 256],
                    in_=ps,
                    func=mybir.ActivationFunctionType.Identity,
                )

            def qkv_v(cc):
                # V in natural [key, vcol] layout straight off the PE:
                # out[key, v] = x^T[kc, key] @ Wv[kc, v], one 128-key block
                # per bank quarter, both heads in the 128 v-columns
                ps = ps_qkv.tile([128, 4, 128], F32, tag="qkv", name="psv")
                for j in range(4):
                    kb = 4 * cc + j
                    for kc in range(4):
                        nc.tensor.matmul(
                            ps[:, j, :],
                            xt_sb[:, kc, cc, j * 128:(j + 1) * 128],
                            wq_sb[:, kc, 256:384],
                            start=(kc == 0),
                            stop=(kc == 3),
                            skip_group_check=True,
                        )
                # v0 -> cols 0:64, v1 -> cols 128:192 of each key block.
                # ACT has early slack, DVE has late slack.
                dst = (vboth[:, 4 * cc:4 * cc + 4, :]
                       .rearrange("p n (g c) -> p n g c", c=64)[:, :, 0:3:2, :])
                src = ps.rearrange("p n (g c) -> p n g c", c=64)
                if cc < 2:
                    nc.scalar.activation(
                        out=dst, in_=src,
                        func=mybir.ActivationFunctionType.Identity,
                    )
                else:
                    nc.vector.tensor_copy(dst, src)

            def scores_block(kb):
                # both heads' score blocks land in the two banks of one st
                # tile so a single exp op covers them (amortizes the ACT
                # PSUM-access bubble)
                ws, we = max(0, kb - 1), min(NB - 1, kb + 1)
                nq = (we - ws + 1) * 128
                moff = (1 - (kb - ws)) * 128
                st = ps_st.tile([128, 2, 512], F32, tag="st", name="st")
                for h in range(2):
                    hp = 64 * h
                    nc.tensor.matmul(
                        st[:, h, :nq],
                        qkvt[hp:hp + 64, 1, kb * 128:(kb + 1) * 128],
                        qkvt[hp:hp + 64, 0, ws * 128:(we + 1) * 128],
                        start=True,
                        stop=True,
                        skip_group_check=True,
                    )
                nc.scalar.activation(
                    out=p_sb[:, kb, 0:2, 0:nq],
                    in_=st[:, 0:2, 0:nq],
                    func=EXPF,
                    bias=km_sb[:, kb:kb + 1],
                    scale=0.125,
                )
                # band mask: only the two 128-col edge triangles of the
                # window need zeroing (the center block is all-ones), read
                # through a strided AP.  Slack-rich masks (consumed several
                # blocks later) go to the otherwise-idle GPSIMD engine;
                # chunk-edge, early, and tail-critical masks stay on DVE.
                eng = (nc.gpsimd
                       if (kb >= 5 and kb % 4 in (1, 2, 3) and kb not in (14, 15))
                       else nc.vector)
                for h in range(2):
                    if kb == 0:           # window [ones | j>=i]
                        pe_ap = p_sb[:, kb, h, 128:256]
                        tm_ap = tm2_sb[:, 256:384]
                    elif kb == NB - 1:    # window [j<=i | ones]
                        pe_ap = p_sb[:, kb, h, 0:128]
                        tm_ap = tm2_sb[:, 0:128]
                    else:                 # [j<=i | ones | j>=i]
                        pe_ap = (p_sb[:, kb, h, 0:384]
                                 .rearrange("p (e c) -> p e c", c=128)
                                 [:, 0:3:2, :])
                        tm_ap = (tm2_sb.rearrange("p (e c) -> p e c", c=128)
                                 [:, 0:3:2, :])
                    eng.tensor_mul(pe_ap, pe_ap, tm_ap)

            ot_live = {}

            def av_mm(h, c, qblo, qbhi, kbs, start, stop, alloc=False):
                # accumulate attn @ [V|ones] for query blocks [qblo, qbhi]
                # into ot(h)'s matching column range (per-element has_written
                # PSUM semantics keep sub-ranges independent)
                if alloc:
                    ot_live[h] = ps_ot.tile([128, 512], F32, tag="ot",
                                            name="ot")
                ot = ot_live[h]
                for i, kb in enumerate(kbs):
                    ws, we = max(0, kb - 1), min(NB - 1, kb + 1)
                    qs, qe = max(ws, qblo), min(we, qbhi)
                    nc.tensor.matmul(
                        ot[:, (qs - 4 * c) * 128:(qe + 1 - 4 * c) * 128],
                        vboth[:, kb, 128 * h:128 * h + 128],
                        p_sb[:, kb, h, (qs - ws) * 128:(qe + 1 - ws) * 128],
                        start=start and i == 0,
                        stop=stop and i == len(kbs) - 1,
                        skip_group_check=True,
                    )

            def av_recip(h, c, qblo, qbhi):
                # reciprocal of the denominator rows (64 identical copies in
                # rows 64:128 of ot) -- emitted right after the AV stop so it
                # runs a block ahead of the normalize multiply
                ot = ot_live[h]
                lo, hi = (qblo - 4 * c) * 128, (qbhi + 1 - 4 * c) * 128
                rcb = small.tile([64, 512], F32R, tag="rc", name="rcb")
                with nc.allow_low_precision("f32r softmax denom recip"):
                    nc.vector.reciprocal(rcb[:, 0:hi - lo], ot[64:128, lo:hi])
                ot_live[h] = (ot, rcb)

            def av_norm(h, c, qblo, qbhi, pop=False):
                hp = 64 * h
                ot, rcb = ot_live.pop(h) if pop else ot_live[h]
                lo, hi = (qblo - 4 * c) * 128, (qbhi + 1 - 4 * c) * 128
                nc.vector.tensor_mul(
                    valst[hp:hp + 64, qblo * 128:(qbhi + 1) * 128],
                    ot[0:64, lo:hi],
                    rcb[:, 0:hi - lo],
                )

            def oproj_range(c, qblo, qbhi, tail=False):
                lo, hi = qblo * 128, (qbhi + 1) * 128
                w = hi - lo
                for fo in range(4):
                    po = ps_qkv.tile([128, 512], F32, tag="qkv", name="po")
                    nc.tensor.matmul(
                        po[:, 0:w],
                        wo_sb[:, fo * 128:(fo + 1) * 128],
                        valst[:, lo:hi],
                        start=True,
                        stop=True,
                    )
                    # the tail hits the region where DVE is the pacer and
                    # ACT idles, so ACT takes all of the copies there
                    on_act = True if tail else fo == 0
                    if on_act:
                        nc.scalar.activation(
                            out=outt_sb[:, fo, lo:hi],
                            in_=po[:, 0:w],
                            func=mybir.ActivationFunctionType.Identity,
                        )
                    else:
                        nc.vector.tensor_copy(outt_sb[:, fo, lo:hi],
                                              po[:, 0:w])
                    # tail: fan the writes over the rings so the final
                    # drains issue in parallel instead of dripping off SP
                    ring = ([nc.sync, nc.scalar, nc.gpsimd, nc.scalar][fo]
                            if tail else nc.sync)
                    ring.dma_start(
                        out=outt[fo * 128:(fo + 1) * 128, lo:hi],
                        in_=outt_sb[:, fo, lo:hi],
                    )

            # chunk c's AV window closes at kb=4c+4; its AV/normalize/o_proj
            # work is spread over kb=4c+4..4c+6 so PE keeps feeding the ACT
            # exp stream with scores while the boundary chain drains.  The
            # final chunk is processed in two 256-query halves: the first
            # half's window closes at kb=14, so only the second half's chain
            # remains after the last exp.
            def kbs_for(c):
                return list(range(max(0, 4 * c - 1), min(NB - 1, 4 * c + 4) + 1))

            for cc in range(4):
                if cc == 0:
                    # first chunk's Q/K in 256-col halves: the first exp only
                    # needs Q blocks 0-1 / K block 0, so fire it early
                    qkv_fb_half(0, 0, 0)
                    qkv_fb_half(0, 1, 0)
                else:
                    qkv_fb(cc, 0)
                    qkv_fb(cc, 1)
                for ikb, kb in enumerate(KB_GROUPS[cc]):
                    scores_block(kb)
                    if ikb == 0:
                        # V projection slots in after the group's first
                        # scores so the exp stream isn't starved at the
                        # chunk boundary
                        if cc == 0:
                            qkv_fb_half(0, 0, 1)
                            qkv_fb_half(0, 1, 1)
                        qkv_v(cc)
                    c = kb // 4 - 1
                    if c >= 0:
                        if kb % 4 == 1:
                            av_mm(0, c, 4 * c, 4 * c + 3, kbs_for(c),
                                  True, True, alloc=True)
                            av_recip(0, c, 4 * c, 4 * c + 3)
                        elif kb % 4 == 2:
                            av_norm(0, c, 4 * c, 4 * c + 3, pop=True)
                            av_mm(1, c, 4 * c, 4 * c + 3, kbs_for(c),
                                  True, True, alloc=True)
                            av_recip(1, c, 4 * c, 4 * c + 3)
                            if kb == 14:
                                # chunk 3 h0: pre-accumulate closed blocks
                                av_mm(0, 3, 12, 15, [11, 12, 13],
                                      True, False, alloc=True)
                        elif kb % 4 == 3:
                            av_norm(1, c, 4 * c, 4 * c + 3, pop=True)
                            oproj_range(c, 4 * c, 4 * c + 3, tail=(c == 2))
            # tail: finish chunk 3
            av_mm(0, 3, 12, 15, [14, 15], False, True)
            av_recip(0, 3, 12, 15)
            av_mm(1, 3, 12, 15, [11, 12, 13, 14, 15], True, True, alloc=True)
            av_recip(1, 3, 12, 15)
            av_norm(0, 3, 12, 15, pop=True)
            av_norm(1, 3, 12, 15, pop=True)
            oproj_range(3, 12, 15, tail=True)

    nc.finalize()
    return nc


def _numpy_reference(x, padding_mask, Wqkv, bqkv, Wo, bo):
    """Fallback for input regimes the device path does not cover."""
    b, s, _ = x.shape
    qkv = x @ Wqkv + bqkv
    qkv = qkv.reshape(b, s, H, 3 * HD).transpose(0, 2, 1, 3)
    q, k, v = np.split(qkv, 3, axis=-1)
    scores = np.einsum("bhqd,bhkd->bhqk", q, k)
    idx = np.arange(s)
    band = np.abs(idx[:, None] - idx[None, :]) <= 128
    pm = padding_mask != 0
    valid = band[None, None] & pm[:, None, None, :] & pm[:, None, :, None]
    scores = np.where(valid, scores, -np.inf) / np.sqrt(HD)
    scores = scores - scores.max(axis=-1, keepdims=True)
    with np.errstate(invalid="ignore", over="ignore"):
        e = np.exp(scores)
        attn = e / e.sum(axis=-1, keepdims=True)
    attn = np.nan_to_num(attn, nan=0.0)
    vals = np.einsum("bhqk,bhkd->bhqd", attn, v)
    vals = vals.transpose(0, 2, 1, 3).reshape(b, s, E)
    return (vals @ Wo + bo).astype(np.float32)


def kernel(x, padding_mask, Wqkv, bqkv, Wo, bo):
    global LAST_RESULTS
    x = np.ascontiguousarray(np.asarray(x, np.float32))
    Wqkv = np.asarray(Wqkv, np.float32)
    bqkv = np.asarray(bqkv, np.float32)
    Wo = np.asarray(Wo, np.float32)
    bo = np.asarray(bo, np.float32)
    pm = np.asarray(padding_mask)

    if np.any(bqkv != 0):
        # qkv bias is identically zero in the target problem; the device
        # program folds no qkv bias, so fall back rather than be wrong.
        return _numpy_reference(x, pm, Wqkv, bqkv, Wo, bo)

    if "nc" not in _CACHE:
        _CACHE["nc"] = _build_nc()
    nc = _CACHE["nc"]

    # trimask [key p, 384]: window cols = [qb-1 | qb | qb+1] relative blocks
    j = np.arange(128)[:, None]
    i = np.arange(128)[None, :]
    tm = np.concatenate(
        [(j <= i), np.ones((128, 128), bool), (j >= i)], axis=1
    ).astype(NPBF)

    in_maps = []
    for core in range(8):
        b, g = divmod(core, 4)
        # feature permutation for this head group: [q0|q1|k0|k1|v0|v1]
        h0, h1 = 2 * g, 2 * g + 1
        cols = []
        for kind in range(3):  # q, k, v
            for h in (h0, h1):
                base = h * 3 * HD + kind * HD
                cols.extend(range(base, base + HD))
        wq_g = Wqkv[:, cols]                                  # [512, 384]
        xt_b = np.ascontiguousarray(x[b].T)                   # [512, 2048]
        xt_cc = np.stack([xt_b[:, cc * 512:(cc + 1) * 512] for cc in range(4)])
        km = np.where(pm[b] != 0, 0.0, -1e5).astype(np.float32)
        in_maps.append({
            "xt": np.ascontiguousarray(xt_cc).astype(NPBF),
            "wq": np.ascontiguousarray(
                wq_g.reshape(4, 128, 384).transpose(1, 0, 2)).astype(NPBF),
            "wo": np.ascontiguousarray(
                Wo[g * 128:(g + 1) * 128, :]).astype(NPBF),
            "km": np.ascontiguousarray(km.reshape(NB, 128).T, dtype=np.float32),
            "tm": tm,
        })

    try:
        LAST_RESULTS = run_bass_kernel_spmd(nc, in_maps, core_ids=list(range(8)))
    except Exception:
        # transient device faults (e.g. NRT_EXEC_UNIT_UNRECOVERABLE) have been
        # observed to clear on the next attempt; retry once before giving up
        LAST_RESULTS = run_bass_kernel_spmd(nc, in_maps, core_ids=list(range(8)))
    res = LAST_RESULTS.results

    out = np.zeros((B, S, E), np.float32)
    for core in range(8):
        b = core // 4
        out[b] += np.asarray(res[core]["outt"]).astype(np.float32).T
    out += bo
    return out


# revision 86
# speedup vs baseline: 1.0104x; 1.0104x over previous
"""Banded (sliding-window) multi-head attention on 8 Trainium2 NeuronCores.

Problem: B=2, S=2048, D=512, H=8 heads (hd=64), window=256 (|i-j| <= 128),
  qkv = x @ Wqkv + bqkv           -> per-head q,k,v
  scores = (q k^T masked to band) / 8 ; softmax ; out = (attn v) @ Wo + bo

Sharding: core = (batch b in {0,1}) x (head-group g in {0..3}); each core
computes 2 heads over the full sequence of one batch element plus the o_proj
partial product for its heads' embed slice. The host sums the 4 partials per
batch and adds bo.

v3: bf16 operands everywhere (fp32 PSUM accumulate), software-pipelined
emission (qkv chunk -> its V dma-transpose -> its scores blocks), o_proj
partials DMA'd straight from PSUM (no copy ops), V transposed by the DMA
xbar instead of the PE. GPSIMD cannot touch PSUM on TRN2, so the PSUM
reads split between ACT (exp, Q/K qkv copies) and DVE (V copies, recip,
fused normalize, band masks).
"""

import numpy as np
import ml_dtypes

import concourse.bass as bass  # noqa: F401  (engine types via nc)
import concourse.mybir as mybir
import concourse.tile as tile
from concourse import bacc
from concourse.bass_utils import run_bass_kernel_spmd

B, S, DIN, E = 2, 2048, 512, 512
H, HD = 8, 64
NB = S // 128      # 16 key/query blocks of 128
NCHUNK = S // 512  # 4 query chunks of 512
F32 = mybir.dt.float32
F32R = mybir.dt.float32r
BF16 = mybir.dt.bfloat16
EXPF = mybir.ActivationFunctionType.Exp
NPBF = ml_dtypes.bfloat16

# kb blocks whose 3-block score window is covered once qkv chunk cc is done
KB_GROUPS = {0: [0, 1, 2], 1: [3, 4, 5, 6], 2: [7, 8, 9, 10],
             3: [11, 12, 13, 14, 15]}

_CACHE = {}
LAST_RESULTS = None  # BassKernelResults of the most recent run (for test.py)


def _build_nc():
    nc = bacc.Bacc(None, target_bir_lowering=False, debug=False)

    xt = nc.dram_tensor("xt", [4, DIN, 512], BF16, kind="ExternalInput")
    wq = nc.dram_tensor("wq", [128, 4, 384], BF16, kind="ExternalInput")
    wo = nc.dram_tensor("wo", [128, E], BF16, kind="ExternalInput")
    km = nc.dram_tensor("km", [128, NB], F32, kind="ExternalInput")
    tm = nc.dram_tensor("tm", [128, 384], BF16, kind="ExternalInput")
    outt = nc.dram_tensor("outt", [E, S], BF16, kind="ExternalOutput")

    with tile.TileContext(nc) as tc:
        with (
            tc.tile_pool(name="sb", bufs=1) as sb,
            tc.tile_pool(name="ps_qkv", bufs=2, space="PSUM") as ps_qkv,
            tc.tile_pool(name="ps_st", bufs=2, space="PSUM") as ps_st,
            tc.tile_pool(name="ps_ot", bufs=2, space="PSUM") as ps_ot,
            tc.tile_pool(name="small", bufs=4) as small,
        ):
            xt_sb = sb.tile([128, 4, 4, 512], BF16)   # [p, kchunk, qchunk, q]
            wq_sb = sb.tile([128, 4, 384], BF16)      # [p, kchunk, fo]
            wo_sb = sb.tile([128, E], BF16)
            km_sb = sb.tile([128, NB], F32)
            tm2_sb = sb.tile([128, 384], BF16)
            qkvt = sb.tile([128, 2, S], BF16)         # fb0=Q, fb1=K (h0|h1)
            # V natural [key, v0|ones|v1|ones]: per head, [v_h | ones] is a
            # contiguous 128-col stationary read, so one AV matmul emits
            # values (rows 0:64) and denominator copies (rows 64:128)
            vboth = sb.tile([128, NB, 256], BF16)
            valst = sb.tile([128, S], BF16)           # normalized attn @ V
            outt_sb = sb.tile([128, 4, S], BF16)
            p_sb = sb.tile([128, NB, 2, 384], BF16)   # exp'd scores, h-minor

            # HWDGE descriptor generation (~630ns/op) serializes the input
            # stream, so only the launch-critical cc0 activations use it; the
            # weights, masks and later x chunks ride the SWDGE ring whose
            # descriptor prep runs on the idle-early Pool engine
            nc.gpsimd.dma_start(out=wq_sb, in_=wq[:, :, :])
            nc.gpsimd.dma_start(out=km_sb, in_=km[:, :])
            nc.gpsimd.dma_start(out=tm2_sb, in_=tm[:, :])
            for half in range(2):
                nc.sync.dma_start(
                    out=xt_sb[:, 2 * half:2 * half + 2, 0, :],
                    in_=xt[0, half * 256:(half + 1) * 256, :]
                    .rearrange("(kc p) q -> p kc q", p=128),
                )
            for cc in range(1, 4):
                nc.gpsimd.dma_start(
                    out=xt_sb[:, :, cc, :],
                    in_=xt[cc].rearrange("(kc p) q -> p kc q", p=128),
                )
            nc.sync.dma_start(out=wo_sb, in_=wo[:, :])

            # denominator ones blocks at cols 64:128 and 192:256 of every key
            # block, so AV emits 64 identical denominator rows (rows 64:128
            # of ot) and the normalize can read the reciprocal per-partition
            # -- no PE broadcast matmul needed
            nc.gpsimd.memset(vboth[:, :, 64:128], 1.0)
            nc.gpsimd.memset(vboth[:, :, 192:256], 1.0)

            def qkv_fb(cc, fb):
                # qkvT = Wg^T @ x[b]^T for query chunk cc, feature block fb;
                # Q/K copies on ACT (they gate the scores stream)
                ps = ps_qkv.tile([128, 512], F32, tag="qkv", name="ps")
                for kc in range(4):
                    nc.tensor.matmul(
                        ps,
                        wq_sb[:, kc, fb * 128:(fb + 1) * 128],
                        xt_sb[:, kc, cc, :],
                        start=(kc == 0),
                        stop=(kc == 3),
                    )
                nc.scalar.activation(
                    out=qkvt[:, fb, cc * 512:(cc + 1) * 512],
                    in_=ps,
                    func=mybir.ActivationFunctionType.Identity,
                )

            def qkv_fb_half(cc, fb, half):
                # 256-col half of a Q/K feature block (ramp shortener)
                o = half * 256
                ps = ps_qkv.tile([128, 256], F32, tag="qkv", name="psh")
                for kc in range(4):
                    nc.tensor.matmul(
                        ps,
                        wq_sb[:, kc, fb * 128:(fb + 1) * 128],
                        xt_sb[:, kc, cc, o:o + 256],
                        start=(kc == 0),
                        stop=(kc == 3),
                    )
                nc.scalar.activation(
                    out=qkvt[:, fb, cc * 512 + o:cc * 512 + o + 256],
                    in_=ps,
                    func=mybir.ActivationFunctionType.Identity,
                )

            def qkv_v(cc):
                # V in natural [key, vcol] layout straight off the PE:
                # out[key, v] = x^T[kc, key] @ Wv[kc, v], one 128-key block
                # per bank quarter, both heads in the 128 v-columns
                ps = ps_qkv.tile([128, 4, 128], F32, tag="qkv", name="psv")
                for j in range(4):
                    kb = 4 * cc + j
                    for kc in range(4):
                        nc.tensor.matmul(
                            ps[:, j, :],
                            xt_sb[:, kc, cc, j * 128:(j + 1) * 128],
                            wq_sb[:, kc, 256:384],
                            start=(kc == 0),
                            stop=(kc == 3),
                            skip_group_check=True,
                        )
                # v0 -> cols 0:64, v1 -> cols 128:192 of each key block.
                # ACT has early slack, DVE has late slack.
                dst = (vboth[:, 4 * cc:4 * cc + 4, :]
                       .rearrange("p n (g c) -> p n g c", c=64)[:, :, 0:3:2, :])
                src = ps.rearrange("p n (g c) -> p n g c", c=64)
                if cc < 2:
                    nc.scalar.activation(
                        out=dst, in_=src,
                        func=mybir.ActivationFunctionType.Identity,
                    )
                else:
                    nc.vector.tensor_copy(dst, src)

            def scores_block(kb):
                # both heads' score blocks land in the two banks of one st
                # tile so a single exp op covers them (amortizes the ACT
                # PSUM-access bubble)
                ws, we = max(0, kb - 1), min(NB - 1, kb + 1)
                nq = (we - ws + 1) * 128
                moff = (1 - (kb - ws)) * 128
                st = ps_st.tile([128, 2, 512], F32, tag="st", name="st")
                for h in range(2):
                    hp = 64 * h
                    nc.tensor.matmul(
                        st[:, h, :nq],
                        qkvt[hp:hp + 64, 1, kb * 128:(kb + 1) * 128],
                        qkvt[hp:hp + 64, 0, ws * 128:(we + 1) * 128],
                        start=True,
                        stop=True,
                        skip_group_check=True,
                    )
                nc.scalar.activation(
                    out=p_sb[:, kb, 0:2, 0:nq],
                    in_=st[:, 0:2, 0:nq],
                    func=EXPF,
                    bias=km_sb[:, kb:kb + 1],
                    scale=0.125,
                )
                # band mask: only the two 128-col edge triangles of the
                # window need zeroing (the center block is all-ones), read
                # through a strided AP.  Slack-rich masks (consumed several
                # blocks later) go to the otherwise-idle GPSIMD engine;
                # chunk-edge, early, and tail-critical masks stay on DVE.
                eng = (nc.gpsimd
                       if (kb >= 5 and kb % 4 in (1, 2, 3) and kb not in (14, 15))
                       else nc.vector)
                for h in range(2):
                    if kb == 0:           # window [ones | j>=i]
                        pe_ap = p_sb[:, kb, h, 128:256]
                        tm_ap = tm2_sb[:, 256:384]
                    elif kb == NB - 1:    # window [j<=i | ones]
                        pe_ap = p_sb[:, kb, h, 0:128]
                        tm_ap = tm2_sb[:, 0:128]
                    else:                 # [j<=i | ones | j>=i]
                        pe_ap = (p_sb[:, kb, h, 0:384]
                                 .rearrange("p (e c) -> p e c", c=128)
                                 [:, 0:3:2, :])
                        tm_ap = (tm2_sb.rearrange("p (e c) -> p e c", c=128)
                                 [:, 0:3:2, :])
                    eng.tensor_mul(pe_ap, pe_ap, tm_ap)

            ot_live = {}

            def av_mm(h, c, qblo, qbhi, kbs, start, stop, alloc=False):
                # accumulate attn @ [V|ones] for query blocks [qblo, qbhi]
                # into ot(h)'s matching column range (per-element has_written
                # PSUM semantics keep sub-ranges independent)
                if alloc:
                    ot_live[h] = ps_ot.tile([128, 512], F32, tag="ot",
                                            name="ot")
                ot = ot_live[h]
                for i, kb in enumerate(kbs):
                    ws, we = max(0, kb - 1), min(NB - 1, kb + 1)
                    qs, qe = max(ws, qblo), min(we, qbhi)
                    nc.tensor.matmul(
                        ot[:, (qs - 4 * c) * 128:(qe + 1 - 4 * c) * 128],
                        vboth[:, kb, 128 * h:128 * h + 128],
                        p_sb[:, kb, h, (qs - ws) * 128:(qe + 1 - ws) * 128],
                        start=start and i == 0,
                        stop=stop and i == len(kbs) - 1,
                        skip_group_check=True,
                    )

            def av_recip(h, c, qblo, qbhi):
                # reciprocal of the denominator rows (64 identical copies in
                # rows 64:128 of ot) -- emitted right after the AV stop so it
                # runs a block ahead of the normalize multiply
                ot = ot_live[h]
                lo, hi = (qblo - 4 * c) * 128, (qbhi + 1 - 4 * c) * 128
                rcb = small.tile([64, 512], F32R, tag="rc", name="rcb")
                with nc.allow_low_precision("f32r softmax denom recip"):
                    nc.vector.reciprocal(rcb[:, 0:hi - lo], ot[64:128, lo:hi])
                ot_live[h] = (ot, rcb)

            def av_norm(h, c, qblo, qbhi, pop=False):
                hp = 64 * h
                ot, rcb = ot_live.pop(h) if pop else ot_live[h]
                lo, hi = (qblo - 4 * c) * 128, (qbhi + 1 - 4 * c) * 128
                nc.vector.tensor_mul(
                    valst[hp:hp + 64, qblo * 128:(qbhi + 1) * 128],
                    ot[0:64, lo:hi],
                    rcb[:, 0:hi - lo],
                )

            def oproj_range(c, qblo, qbhi, tail=False):
                lo, hi = qblo * 128, (qbhi + 1) * 128
                w = hi - lo
                for fo in range(4):
                    po = ps_qkv.tile([128, 512], F32, tag="qkv", name="po")
                    nc.tensor.matmul(
                        po[:, 0:w],
                        wo_sb[:, fo * 128:(fo + 1) * 128],
                        valst[:, lo:hi],
                        start=True,
                        stop=True,
                    )
                    # c=2 lands where DVE still runs its normalize chain, so
                    # ACT takes all of its copies; c=3's copies split so the
                    # final drain uses both engines
                    if tail:
                        on_act = True if c == 2 else fo in (0, 2)
                    else:
                        on_act = fo == 0
                    if on_act:
                        nc.scalar.activation(
                            out=outt_sb[:, fo, lo:hi],
                            in_=po[:, 0:w],
                            func=mybir.ActivationFunctionType.Identity,
                        )
                    else:
                        nc.vector.tensor_copy(outt_sb[:, fo, lo:hi],
                                              po[:, 0:w])
                    # tail: fan the writes over the rings so the final
                    # drains issue in parallel instead of dripping off SP
                    ring = ([nc.sync, nc.scalar, nc.sync, nc.scalar][fo]
                            if tail else nc.sync)
                    ring.dma_start(
                        out=outt[fo * 128:(fo + 1) * 128, lo:hi],
                        in_=outt_sb[:, fo, lo:hi],
                    )

            # chunk c's AV window closes at kb=4c+4; its AV/normalize/o_proj
            # work is spread over kb=4c+4..4c+6 so PE keeps feeding the ACT
            # exp stream with scores while the boundary chain drains.  The
            # final chunk is processed in two 256-query halves: the first
            # half's window closes at kb=14, so only the second half's chain
            # remains after the last exp.
            def kbs_for(c):
                return list(range(max(0, 4 * c - 1), min(NB - 1, 4 * c + 4) + 1))

            for cc in range(4):
                if cc == 0:
                    # first chunk's Q/K in 256-col halves: the first exp only
                    # needs Q blocks 0-1 / K block 0, so fire it early
                    qkv_fb_half(0, 0, 0)
                    qkv_fb_half(0, 1, 0)
                else:
                    qkv_fb(cc, 0)
                    qkv_fb(cc, 1)
                for ikb, kb in enumerate(KB_GROUPS[cc]):
                    scores_block(kb)
                    if ikb == 0:
                        # V projection slots in after the group's first
                        # scores so the exp stream isn't starved at the
                        # chunk boundary
                        if cc == 0:
                            qkv_fb_half(0, 0, 1)
                            qkv_fb_half(0, 1, 1)
                        qkv_v(cc)
                    c = kb // 4 - 1
                    if c >= 0:
                        if kb % 4 == 1:
                            av_mm(0, c, 4 * c, 4 * c + 3, kbs_for(c),
                                  True, True, alloc=True)
                            av_recip(0, c, 4 * c, 4 * c + 3)
                        elif kb % 4 == 2:
                            av_norm(0, c, 4 * c, 4 * c + 3, pop=True)
                            av_mm(1, c, 4 * c, 4 * c + 3, kbs_for(c),
                                  True, True, alloc=True)
                            av_recip(1, c, 4 * c, 4 * c + 3)
                            if kb == 14:
                                # chunk 3 h0: pre-accumulate closed blocks
                                av_mm(0, 3, 12, 15, [11, 12, 13],
                                      True, False, alloc=True)
                        elif kb % 4 == 3:
                            av_norm(1, c, 4 * c, 4 * c + 3, pop=True)
                            oproj_range(c, 4 * c, 4 * c + 3, tail=(c == 2))
            # tail: finish chunk 3
            av_mm(0, 3, 12, 15, [14, 15], False, True)
            av_recip(0, 3, 12, 15)
            av_mm(1, 3, 12, 15, [11, 12, 13, 14, 15], True, True, alloc=True)
            av_recip(1, 3, 12, 15)
            av_norm(0, 3, 12, 15, pop=True)
            av_norm(1, 3, 12, 15, pop=True)
            oproj_range(3, 12, 15, tail=True)

    nc.finalize()
    return nc


def _numpy_reference(x, padding_mask, Wqkv, bqkv, Wo, bo):
    """Fallback for input regimes the device path does not cover."""
    b, s, _ = x.shape
    qkv = x @ Wqkv + bqkv
    qkv = qkv.reshape(b, s, H, 3 * HD).transpose(0, 2, 1, 3)
    q, k, v = np.split(qkv, 3, axis=-1)
    scores = np.einsum("bhqd,bhkd->bhqk", q, k)
    idx = np.arange(s)
    band = np.abs(idx[:, None] - idx[None, :]) <= 128
    pm = padding_mask != 0
    valid = band[None, None] & pm[:, None, None, :] & pm[:, None, :, None]
    scores = np.where(valid, scores, -np.inf) / np.sqrt(HD)
    scores = scores - scores.max(axis=-1, keepdims=True)
    with np.errstate(invalid="ignore", over="ignore"):
        e = np.exp(scores)
        attn = e / e.sum(axis=-1, keepdims=True)
    attn = np.nan_to_num(attn, nan=0.0)
    vals = np.einsum("bhqk,bhkd->bhqd", attn, v)
    vals = vals.transpose(0, 2, 1, 3).reshape(b, s, E)
    return (vals @ Wo + bo).astype(np.float32)


def kernel(x, padding_mask, Wqkv, bqkv, Wo, bo):
    global LAST_RESULTS
    x = np.ascontiguousarray(np.asarray(x, np.float32))
    Wqkv = np.asarray(Wqkv, np.float32)
    bqkv = np.asarray(bqkv, np.float32)
    Wo = np.asarray(Wo, np.float32)
    bo = np.asarray(bo, np.float32)
    pm = np.asarray(padding_mask)

    if np.any(bqkv != 0):
        # qkv bias is identically zero in the target problem; the device
        # program folds no qkv bias, so fall back rather than be wrong.
        return _numpy_reference(x, pm, Wqkv, bqkv, Wo, bo)

    if "nc" not in _CACHE:
        _CACHE["nc"] = _build_nc()
    nc = _CACHE["nc"]

    # trimask [key p, 384]: window cols = [qb-1 | qb | qb+1] relative blocks
    j = np.arange(128)[:, None]
    i = np.arange(128)[None, :]
    tm = np.concatenate(
        [(j <= i), np.ones((128, 128), bool), (j >= i)], axis=1
    ).astype(NPBF)

    in_maps = []
    for core in range(8):
        b, g = divmod(core, 4)
        # feature permutation for this head group: [q0|q1|k0|k1|v0|v1]
        h0, h1 = 2 * g, 2 * g + 1
        cols = []
        for kind in range(3):  # q, k, v
            for h in (h0, h1):
                base = h * 3 * HD + kind * HD
                cols.extend(range(base, base + HD))
        wq_g = Wqkv[:, cols]                                  # [512, 384]
        xt_b = np.ascontiguousarray(x[b].T)                   # [512, 2048]
        xt_cc = np.stack([xt_b[:, cc * 512:(cc + 1) * 512] for cc in range(4)])
        km = np.where(pm[b] != 0, 0.0, -1e5).astype(np.float32)
        in_maps.append({
            "xt": np.ascontiguousarray(xt_cc).astype(NPBF),
            "wq": np.ascontiguousarray(
                wq_g.reshape(4, 128, 384).transpose(1, 0, 2)).astype(NPBF),
            "wo": np.ascontiguousarray(
                Wo[g * 128:(g + 1) * 128, :]).astype(NPBF),
            "km": np.ascontiguousarray(km.reshape(NB, 128).T, dtype=np.float32),
            "tm": tm,
        })

    try:
        LAST_RESULTS = run_bass_kernel_spmd(nc, in_maps, core_ids=list(range(8)))
    except Exception:
        # transient device faults (e.g. NRT_EXEC_UNIT_UNRECOVERABLE) have been
        # observed to clear on the next attempt; retry once before giving up
        LAST_RESULTS = run_bass_kernel_spmd(nc, in_maps, core_ids=list(range(8)))
    res = LAST_RESULTS.results

    out = np.zeros((B, S, E), np.float32)
    for core in range(8):
        b = core // 4
        out[b] += np.asarray(res[core]["outt"]).astype(np.float32).T
    out += bo
    return out
